# revision 1
# baseline (speedup 1.0000x reference)
"""Trainium2 Bass kernel for nn_DBLossWithShift.

Computes: mean((y_hat-y)^2) + 0.1 * min_{|d|<=5} mean((EMA(y_hat)[t+d]-EMA(y)[t])^2)
for y_hat, y of shape [128, 8192, 8] f32, EMA along t with alpha=0.2.

Strategy (data-parallel over batch, 8 cores x 16 batch elements):
  Host: per-core shard -> transpose to [t, (b,c)] layout -> cast bf16.
  Device (per core), per 128-t block j (64 blocks, grouped 4 per SBUF tile):
    - EMA as matmul: E^T[bc, t_out] = sum_t_in X[t_in, bc] * A[t_in, t_out]
      with A the (truncated, 2-block-banded) EMA coefficient matrix.
      Accumulated in PSUM (Aprev writes, Acur accumulates).
    - PSUM -> SBUF bf16 copies of E_hat^T, E^T (ScalarE).
    - corr Gram: G[t',t] += sum_bc Eh^T[bc,t'] * Ee^T[bc,t] (TensorE, PSUM acc).
    - db partial: sum((xh-x)^2) via DVE sub + tensor_tensor_reduce.
    - sum(D^2), D = Eh-Ee, same way.
  Host: assemble losses in f64; shifted-MSE decomposition:
      err(d)*N_d = (sum Eh^2 + sum Ee^2) - head/tail edges - 2*corr(d)
      sum Eh^2 + sum Ee^2 = sum D^2 + 2*corr(0)
      corr(d) = sum of diag(G, -d)   (block-boundary cross terms dropped;
                                      contributes O(1e-4) relative on the
                                      shift loss, O(1e-6) on the total)
      edges computed exactly on host from the raw f32 inputs.
"""

import sys

import numpy as np

for _p in ("/opt/trn_rl_repo",):
    if _p not in sys.path:
        sys.path.insert(0, _p)

import ml_dtypes

# ---------------------------------------------------------------- constants
B, T, C = 128, 8192, 8
NCORES = 8
BPC = B // NCORES          # 16 batch elements per core
P = 128                    # t-block size (partition dim)
NBLK = T // P              # 64 blocks
GRP = 4                    # blocks per SBUF/PSUM group tile
NGRP = NBLK // GRP         # 16 groups
GW = GRP * P               # 512, group tile free width
BC = BPC * C               # 128 channels per core (b*8 + c)
ALPHA = 0.2
LAM = 0.1
KSH = 5                    # max |shift|

# Which engine computes the two sum-of-squares reductions after the DVE sub:
# "dve": tensor_tensor_reduce (broken on current HW - crashes the exec unit)
# "pe":  per-slot self-Gram matmuls accumulated in PSUM, diag read on host
# "act": ScalarE activation(Square) with accum_out
DB_MODE = "act"
D2_MODE = "pe"
# Fuse the two EMA matmuls per x-slot into one N=256 matmul ([Acur|Aprev] rhs)
# for slots 0-2. Correct on HW (per-element has_written) but trips CoreSim's
# zero-region uniformity assert, so keep False when running `test.py sim`.
FUSE_EMA = False
# Truncate Aprev to its 64 nonneg-significant columns (gap>64 coefficients are
# < 1e-7 relative) and split Acur into two 64-col matmuls so each matmul's
# psum region stays uniformly overwrite-or-accumulate (CoreSim-safe).
EMA_SPLIT64 = False
WPREV = 64
# Halve the granularity of the PSUM->SBUF evacuations / input DMAs so
# downstream consumers (Gram matmuls / EMA matmuls) start earlier.
SPLIT_EVAC = False
SPLIT_DMA = False
# How many 4-block groups each input DMA carries (1, 2, 4, 8, or 16).
# Bigger -> fewer DMA fixed costs; host layout places them contiguously.
GROUPS_PER_DMA = 1
# Which engine evacuates each EMA PSUM group tile to SBUF bf16.
EVAC_EH = "act"
EVAC_EE = "dve"
PGRP_BUFS = 2
XPOOL_BUFS = 4
EPOOL_BUFS = 4
DPOOL_BUFS = 3

_BF16 = ml_dtypes.bfloat16


def _ema_mats():
    """A matrices (f64): E^T = X_prev^T@Aprev + X_cur^T@Acur, [t_in, t_out]."""
    a = ALPHA
    q = 1.0 - a
    t_in = np.arange(P)[:, None]
    t_out = np.arange(P)[None, :]
    gap = t_out - t_in
    acur = np.where(gap >= 0, a * q ** np.clip(gap, 0, None), 0.0)
    a0 = acur.copy()
    a0[0, :] = q ** t_out[0]          # e_0 = x_0 boundary: coeff of x_0 is q^t
    gapp = t_out + P - t_in           # in [1, 255]
    aprev = a * q ** gapp
    return a0, acur, aprev


# ---------------------------------------------------------------- device IR
_MODULE_CACHE = {}


def _build_module():
    if "nc" in _MODULE_CACHE:
        return _MODULE_CACHE["nc"]
    from contextlib import ExitStack

    import concourse.tile as tile
    from concourse import bacc, mybir

    f32 = mybir.dt.float32
    bf16 = mybir.dt.bfloat16
    alu = mybir.AluOpType

    a0_np, acur_np, aprev_np = _ema_mats()

    gpd = GROUPS_PER_DMA
    nsuper = NGRP // gpd
    nc = bacc.Bacc("TRN2", target_bir_lowering=False, debug=False)
    yh_d = nc.dram_tensor("yh", [nsuper, P, gpd * GW], bf16, kind="ExternalInput")
    yy_d = nc.dram_tensor("yy", [nsuper, P, gpd * GW], bf16, kind="ExternalInput")
    # single packed output: [stage(32) | gmid(128) | gd2(128)? | gdb(128)?]
    out_w = 2 * NGRP + P + (P if D2_MODE == "pe" else 0) \
        + (P if DB_MODE == "pe" else 0)
    out_d = nc.dram_tensor("out", [P, out_w], f32, kind="ExternalOutput")

    allc_c = nc.inline_tensor(
        np.concatenate([a0_np, acur_np, aprev_np, acur_np, aprev_np,
                        a0_np, aprev_np], axis=1).astype(_BF16), "allmat")

    yh_ap, yy_ap = yh_d.ap(), yy_d.ap()

    with tile.TileContext(nc) as tc, ExitStack() as ctx:
        consts = ctx.enter_context(tc.tile_pool(name="consts", bufs=1))
        xpool = ctx.enter_context(tc.tile_pool(name="xin", bufs=XPOOL_BUFS))
        epool = ctx.enter_context(tc.tile_pool(name="ecopy", bufs=EPOOL_BUFS))
        dpool = ctx.enter_context(tc.tile_pool(name="dtmp", bufs=DPOOL_BUFS))
        opool = ctx.enter_context(tc.tile_pool(name="outs", bufs=1))
        pgrp = ctx.enter_context(
            tc.tile_pool(name="pgrp", bufs=PGRP_BUFS, space="PSUM"))
        pacc = ctx.enter_context(tc.tile_pool(name="pacc", bufs=1, space="PSUM"))

        allc_t = consts.tile([P, 7 * P], bf16)
        nc.sync.dma_start(allc_t[:], allc_c.ap())
        a0_t = allc_t[:, 0:P]
        ac_t = allc_t[:, P:2 * P]
        ap_t = allc_t[:, 2 * P:3 * P]
        acp_t = allc_t[:, 3 * P:5 * P]   # [Acur | Aprev]
        a0p_t = allc_t[:, 5 * P:7 * P]   # [A0 | Aprev]

        gmid = pacc.tile([P, P], f32)
        if DB_MODE == "pe":
            gdb = pacc.tile([P, P], f32)
        if D2_MODE == "pe":
            gd2 = pacc.tile([P, P], f32)
        out_s = opool.tile([P, out_w], f32)
        stage_t = out_s[:, 0:2 * NGRP]
        if DB_MODE == "pe" or D2_MODE == "pe":
            nc.vector.memset(stage_t[:], 0.0)

        ph_next = pe_next = None
        xh_sup = xx_sup = None
        for g in range(NGRP):
            sg, gg = divmod(g, gpd)
            if gg == 0:
                xh_sup = xpool.tile([P, gpd * GW], bf16, tag="xh")
                nc.sync.dma_start(xh_sup[:], yh_ap[sg])
                xx_sup = xpool.tile([P, gpd * GW], bf16, tag="xx")
                nc.sync.dma_start(xx_sup[:], yy_ap[sg])
            xh = xh_sup[:, gg * GW:(gg + 1) * GW]
            xx = xx_sup[:, gg * GW:(gg + 1) * GW]

            ph = ph_next if ph_next is not None else pgrp.tile([P, GW], f32, tag="ph")
            pe = pe_next if pe_next is not None else pgrp.tile([P, GW], f32, tag="pe")
            if g + 1 < NGRP:
                ph_next = pgrp.tile([P, GW], f32, tag="ph")
                pe_next = pgrp.tile([P, GW], f32, tag="pe")
            else:
                ph_next = pe_next = None

            # --- EMA matmuls. Per slot s the psum region [s*P,(s+1)*P) gets:
            #   Aprev write (from x slot s-1, pending-zero -> overwrite), then
            #   Acur accumulate (from x slot s).
            # Emission order keeps consecutive matmuls sharing lhsT.
            for xt, ps, ps_next in ((xh, ph, ph_next), (xx, pe, pe_next)):
                for s in range(GRP):
                    j = g * GRP + s
                    lhs = xt[:, s * P:(s + 1) * P]
                    if FUSE_EMA and s < GRP - 1:
                        rhs2 = a0p_t if j == 0 else acp_t
                        nc.tensor.matmul(
                            ps[:, s * P:(s + 2) * P], lhs, rhs2,
                            start=(j == 0), stop=False,
                        )
                        continue
                    if EMA_SPLIT64 and j > 0:
                        o = s * P
                        # Aprev (from x_{j-1}, emitted last iter) wrote cols
                        # [o, o+WPREV): accumulate there, overwrite the rest.
                        nc.tensor.matmul(
                            ps[:, o:o + WPREV], lhs, ac_t[:, 0:WPREV],
                            start=False, stop=False,
                        )
                        nc.tensor.matmul(
                            ps[:, o + WPREV:o + P], lhs, ac_t[:, WPREV:P],
                            start=False, stop=(s == GRP - 1),
                        )
                    else:
                        rcur = a0_t if j == 0 else ac_t
                        nc.tensor.matmul(
                            ps[:, s * P:(s + 1) * P], lhs, rcur,
                            start=(j == 0), stop=(s == GRP - 1),
                        )
                    if j < NBLK - 1:
                        wp = WPREV if EMA_SPLIT64 else P
                        if s < GRP - 1:
                            nc.tensor.matmul(
                                ps[:, (s + 1) * P:(s + 1) * P + wp],
                                lhs, ap_t[:, 0:wp],
                                start=False, stop=False,
                            )
                        else:
                            nc.tensor.matmul(
                                ps_next[:, 0:wp], lhs, ap_t[:, 0:wp],
                                start=True, stop=False,
                            )

            # --- evacuate EMA to SBUF bf16
            eh = epool.tile([P, GW], bf16, tag="eh")
            ee = epool.tile([P, GW], bf16, tag="ee")
            for dst, src, eng in ((eh, ph, EVAC_EH), (ee, pe, EVAC_EE)):
                op = nc.scalar.copy if eng == "act" else nc.vector.tensor_copy
                if SPLIT_EVAC:
                    h = GW // 2
                    op(dst[:, 0:h], src[:, 0:h])
                    op(dst[:, h:GW], src[:, h:GW])
                else:
                    op(dst[:], src[:])

            # --- corr Gram accumulation: G[t',t] += sum_bc Eh^T Ee^T
            for s in range(GRP):
                j = g * GRP + s
                nc.tensor.matmul(
                    gmid[:], eh[:, s * P:(s + 1) * P], ee[:, s * P:(s + 1) * P],
                    start=(j == 0), stop=(j == NBLK - 1),
                )

            # --- db partial: sum over group of (xh-x)^2
            diff = dpool.tile([P, GW], bf16, tag="diff")
            nc.vector.tensor_sub(diff[:], xh[:], xx[:])
            if DB_MODE == "dve":
                junk = dpool.tile([P, GW], bf16, tag="junk")
                nc.vector.tensor_tensor_reduce(
                    junk[:], diff[:], diff[:], 1.0, 0.0,
                    alu.mult, alu.add, stage_t[:, g:g + 1],
                )
            elif DB_MODE == "act":
                junk = dpool.tile([P, GW], bf16, tag="junk")
                nc.scalar.activation(
                    junk[:], diff[:], mybir.ActivationFunctionType.Square,
                    accum_out=stage_t[:, g:g + 1],
                )
            else:
                for s in range(GRP):
                    sl = diff[:, s * P:(s + 1) * P]
                    nc.tensor.matmul(
                        gdb[:], sl, sl,
                        start=(g == 0 and s == 0),
                        stop=(g == NGRP - 1 and s == GRP - 1),
                    )

            # --- sum(D^2), D = Eh - Ee
            dd = dpool.tile([P, GW], bf16, tag="dd")
            nc.vector.tensor_sub(dd[:], eh[:], ee[:])
            if D2_MODE == "dve":
                junk2 = dpool.tile([P, GW], bf16, tag="junk2")
                nc.vector.tensor_tensor_reduce(
                    junk2[:], dd[:], dd[:], 1.0, 0.0,
                    alu.mult, alu.add, stage_t[:, NGRP + g:NGRP + g + 1],
                )
            elif D2_MODE == "act":
                junk2 = dpool.tile([P, GW], bf16, tag="junk2")
                nc.scalar.activation(
                    junk2[:], dd[:], mybir.ActivationFunctionType.Square,
                    accum_out=stage_t[:, NGRP + g:NGRP + g + 1],
                )
            else:
                for s in range(GRP):
                    sl = dd[:, s * P:(s + 1) * P]
                    nc.tensor.matmul(
                        gd2[:], sl, sl,
                        start=(g == 0 and s == 0),
                        stop=(g == NGRP - 1 and s == GRP - 1),
                    )

        # --- outputs
        col = 2 * NGRP
        nc.vector.tensor_copy(out_s[:, col:col + P], gmid[:])
        col += P
        if D2_MODE == "pe":
            nc.vector.tensor_copy(out_s[:, col:col + P], gd2[:])
            col += P
        if DB_MODE == "pe":
            nc.vector.tensor_copy(out_s[:, col:col + P], gdb[:])
            col += P
        nc.sync.dma_start(out_d.ap(), out_s[:])

    nc.compile()
    _MODULE_CACHE["nc"] = nc
    return nc


# ---------------------------------------------------------------- host side
def _shard_core(y_hat, y, core):
    """Per-core [16,8192,8] f32 -> [16, 128, 512] bf16 in (g, tf, blk, bc)."""
    gpd = GROUPS_PER_DMA
    nsuper = NGRP // gpd
    outs = []
    for arr in (y_hat, y):
        s = arr[core * BPC:(core + 1) * BPC]                # [16, T, 8]
        x = s.transpose(1, 0, 2).reshape(T, BC)             # [t, bc]
        x = x.reshape(nsuper, gpd, GRP, P, BC)              # (sg, gg, blk, tf, bc)
        x = x.transpose(0, 3, 1, 2, 4).reshape(nsuper, P, gpd * GW)
        outs.append(np.ascontiguousarray(x).astype(_BF16))
    return outs


def _emulate_core(yh_g, yy_g):
    """Numpy emulation of the device kernel for one core (validation aid).

    yh_g, yy_g: [NGRP, P, GW] bf16. Returns dict like the device outputs.
    """
    a0_np, acur_np, aprev_np = _ema_mats()
    a0 = a0_np.astype(_BF16).astype(np.float32)
    ac = acur_np.astype(_BF16).astype(np.float32)
    ap = aprev_np.astype(_BF16).astype(np.float32)

    def blocks(xg):
        # -> [NBLK, P(t), BC]
        gpd = GROUPS_PER_DMA
        x = xg.astype(np.float32).reshape(NGRP // gpd, P, gpd, GRP, BC)
        return x.transpose(0, 2, 3, 1, 4).reshape(NBLK, P, BC)

    xh_b, xx_b = blocks(yh_g), blocks(yy_g)

    def ema(xb):
        e = np.zeros((NBLK, P, BC), np.float32)  # e[j] = E block j, [t, bc]
        for j in range(NBLK):
            m = a0 if j == 0 else ac
            acc = m.T @ xb[j]
            if j > 0:
                acc = acc + ap.T @ xb[j - 1]
            e[j] = acc
        return e

    eh_b = ema(xh_b)   # f32 accumulation like PSUM
    ee_b = ema(xx_b)
    # bf16 storage of E (the ACT copies)
    eh_s = eh_b.astype(_BF16).astype(np.float32)
    ee_s = ee_b.astype(_BF16).astype(np.float32)

    gmid = np.zeros((P, P), np.float64)
    for j in range(NBLK):
        # device: G[t',t] += sum_bc Eh[t',bc] * Ee[t,bc]
        gmid += eh_s[j].astype(np.float64) @ ee_s[j].astype(np.float64).T

    diff = (xh_b.astype(_BF16).astype(np.float64)
            - xx_b.astype(_BF16).astype(np.float64)).astype(_BF16).astype(np.float64)
    db = float((diff ** 2).sum())
    dd = (eh_s - ee_s).astype(_BF16).astype(np.float64)
    d2 = float((dd ** 2).sum())
    return {"gmid": gmid.astype(np.float32),
            "db": db, "d2": d2}


def _host_edges(y_hat, y):
    """Exact (f64) head/tail EMA values of the full batch.

    Returns (head_h, head_e, tail_h, tail_e): each [KSH, B, C] f64 where
    head_*[k] = EMA[t=k], tail_*[k] = EMA[t=T-KSH+k].
    """
    a, q = ALPHA, 1.0 - ALPHA
    heads, tails = [], []
    for arr in (y_hat, y):
        x = arr.astype(np.float64)
        e = x[:, 0, :]
        hh = [e]
        for t in range(1, KSH):
            e = a * x[:, t, :] + q * e
            hh.append(e)
        heads.append(np.stack(hh))
        wtail = 700
        e = np.zeros_like(x[:, 0, :])
        tt = {}
        for t in range(T - wtail, T):
            e = a * x[:, t, :] + q * e
            if t >= T - KSH:
                tt[t] = e
        tails.append(np.stack([tt[T - KSH + k] for k in range(KSH)]))
    return heads[0], heads[1], tails[0], tails[1]


def _host_reduce(gmid_sum, db_num, d2_num, y_hat, y):
    """Assemble the final scalar loss (f64)."""
    head_h, head_e, tail_h, tail_e = _host_edges(y_hat, y)
    hh2 = (head_h ** 2).sum(axis=(1, 2))   # [KSH] per-t sums
    he2 = (head_e ** 2).sum(axis=(1, 2))
    th2 = (tail_h ** 2).sum(axis=(1, 2))
    te2 = (tail_e ** 2).sum(axis=(1, 2))

    corr = {d: np.diagonal(gmid_sum, offset=-d).sum() for d in range(-KSH, KSH + 1)}
    corr0 = corr[0]
    normsum = d2_num + 2.0 * corr0

    errs = []
    for d in range(-KSH, KSH + 1):
        nd = B * C * (T - abs(d))
        if d >= 0:
            # pred Eh[t'] for t' in [d, T); target Ee[t] for t in [0, T-d)
            head_cut = hh2[:d].sum() if d > 0 else 0.0
            tail_cut = te2[KSH - d:].sum() if d > 0 else 0.0
        else:
            s = -d
            head_cut = he2[:s].sum()
            tail_cut = th2[KSH - s:].sum()
        num = normsum - head_cut - tail_cut - 2.0 * corr[d]
        errs.append(num / nd)

    db_loss = db_num / (B * T * C)
    return db_loss + LAM * min(errs)


def _unpack_out(out):
    parts = {"stage": out[:, 0:2 * NGRP]}
    col = 2 * NGRP
    parts["gmid"] = out[:, col:col + P]
    col += P
    if D2_MODE == "pe":
        parts["gd2"] = out[:, col:col + P]
        col += P
    if DB_MODE == "pe":
        parts["gdb"] = out[:, col:col + P]
    return parts


def _run_device(y_hat, y, trace=False):
    """Build shards, run the SPMD kernel, return per-core result dicts."""
    from concourse.bass_utils import run_bass_kernel_spmd

    nc = _build_module()
    in_maps = []
    for core in range(NCORES):
        yh_g, yy_g = _shard_core(y_hat, y, core)
        in_maps.append({"yh": yh_g, "yy": yy_g})
    res = run_bass_kernel_spmd(
        nc, in_maps, core_ids=list(range(NCORES)), trace=trace,
    )
    return res


def _reduce_results(results, y_hat, y):
    gmid_sum = np.zeros((P, P), np.float64)
    db_num = 0.0
    d2_num = 0.0
    for r in results:
        parts = _unpack_out(r["out"])
        gmid_sum += parts["gmid"].astype(np.float64)
        if DB_MODE in ("dve", "act"):
            db_num += parts["stage"][:, :NGRP].astype(np.float64).sum()
        else:
            db_num += np.trace(parts["gdb"].astype(np.float64))
        if D2_MODE in ("dve", "act"):
            d2_num += parts["stage"][:, NGRP:].astype(np.float64).sum()
        else:
            d2_num += np.trace(parts["gd2"].astype(np.float64))
    out = _host_reduce(gmid_sum, db_num, d2_num, y_hat, y)
    return np.float32(out)


def kernel(y_hat, y):
    res = _run_device(y_hat, y, trace=False)
    return _reduce_results(res.results, y_hat, y)



# revision 2
# speedup vs baseline: 2.6052x; 2.6052x over previous
"""Trainium2 Bass kernel for nn_DBLossWithShift.

Computes: mean((y_hat-y)^2) + 0.1 * min_{|d|<=5} mean((EMA(y_hat)[t+d]-EMA(y)[t])^2)
for y_hat, y of shape [128, 8192, 8] f32, EMA along t with alpha=0.2.

Key identity: the EMA is a linear filter, so every term of the loss is a
quadratic form in the raw inputs.  With the stationary kernel a[i] = a*q^i,

    sum_t E1[t+d]*E2[t]  =  sum_s K(s-d) * R_12(s),    K(m) = a^2 q^|m| / (1-q^2)

where R_12(s) = sum_t x1[t+s]*x2[t] is the raw lag-s cross-correlation.  The
device therefore only computes three block-diagonal Gram matrices of the raw
(fp8-quantized) inputs per core:

    G_12[u, v] = sum_{j<64} sum_{bc} x1[bc, 128j+u] * x2[bc, 128j+v]

for (x_hat,x), (x_hat,x_hat), (x,x).  Host sums the G's over cores, takes
diagonal sums R~(s), applies the K smoothing, and adds exact f64 head/tail
corrections (EMA init e_0 = x_0, range cuts, beyond-T decay) computed from
short scans of the raw inputs.  db_loss comes exactly from the s=0 diagonals.

Approximations (validated ~7.6e-4 total rel err vs reference, gate is 2e-2):
  - fp8_e4m3 input quantization (dominant: ~7.6e-4 on db)
  - cross-block lag pairs dropped from R~ (~1e-6)
  - K truncated at |m| <= S_BAND=96 (~1e-9)

Device schedule (per core): 9 input DMA chunks (fp8, 2 MiB total) overlapped
with 96 DoubleRow fp8 matmuls (K=256 via block pairs) accumulating the three
Grams in one PSUM bank; one DVE evacuation to bf16; one output DMA.
Data-parallel over batch: 16 batch elements per core, (b,c) -> 128 partitions.
"""

import sys

import numpy as np

for _p in ("/opt/trn_rl_repo",):
    if _p not in sys.path:
        sys.path.insert(0, _p)

import ml_dtypes

# ---------------------------------------------------------------- constants
B, T, C = 128, 8192, 8
NCORES = 8
BPC = B // NCORES          # 16 batch elements per core
BC = BPC * C               # 128 partitions (b*8 + c)
P = 128                    # t-block size
NBLK = T // P              # 64 blocks
ALPHA, QD = 0.2, 0.8
KSH = 5                    # max |shift|
LAM = 0.1

S_BAND = 96                # diagonal band of G used by the host reduction
H_HEAD = 192               # head-scan length for exact EMA-init corrections
EXT = 224                  # tail extension (decay) length
TAIL_WARM = 768            # tail-scan warmup steps

# input DMA chunk sizes in blocks (sum = 64, all even)
CHUNKS = (8, 8, 8, 8, 8, 8, 8, 6, 2)

FP8 = ml_dtypes.float8_e4m3    # TRN float8e4 (max +-240; N(0,1) data is safe)


# ---------------------------------------------------------------- device IR
_MODULE_CACHE = {}


def _build_module():
    if "nc" in _MODULE_CACHE:
        return _MODULE_CACHE["nc"]
    from contextlib import ExitStack

    import concourse.tile as tile
    from concourse import bacc, mybir

    f8 = mybir.dt.float8e4
    f32 = mybir.dt.float32
    bf16 = mybir.dt.bfloat16
    DR = mybir.MatmulPerfMode.DoubleRow

    nc = bacc.Bacc("TRN2", target_bir_lowering=False, debug=False)
    # X layout: [bc, blk, tensor, t] with tensor 0 = y (xe), 1 = y_hat (xh)
    x_d = nc.dram_tensor("x", [BC, NBLK, 2, P], f8, kind="ExternalInput")
    # out: [he | hh | ee] Grams, bf16
    out_d = nc.dram_tensor("out", [P, 3 * P], bf16, kind="ExternalOutput")

    with tile.TileContext(nc) as tc, ExitStack() as ctx:
        xp = ctx.enter_context(tc.tile_pool(name="xin", bufs=1))
        op = ctx.enter_context(tc.tile_pool(name="outs", bufs=1))
        pp = ctx.enter_context(tc.tile_pool(name="pacc", bufs=1, space="PSUM"))

        xt = xp.tile([BC, NBLK, 2, P], f8, tag="xt")
        c0 = 0
        for i, cb in enumerate(CHUNKS):
            c1 = c0 + cb
            eng = nc.sync if i % 2 == 0 else nc.scalar
            eng.dma_start(xt[:, c0:c1], x_d.ap()[:, c0:c1])
            c0 = c1
        assert c0 == NBLK

        g = pp.tile([P, 3 * P], f32, tag="g")
        os_ = op.tile([P, 3 * P], bf16, tag="os")

        # One accumulation group for the whole PSUM bank: start only on the
        # very first matmul (marks the 2 KiB zero region pending-zero once),
        # stop only on the last.  Each matmul's 128-col slice is uniformly
        # virgin (pair 0: overwrite via has_written) or written (accumulate).
        npair = NBLK // 2
        for m in range(npair):
            j0 = 2 * m
            lh = xt[:, j0:j0 + 2, 1, :]
            le = xt[:, j0:j0 + 2, 0, :]
            first = m == 0
            last = m == npair - 1
            nc.tensor.matmul(g[:, 0:P], lh, le,
                             start=first, stop=False, perf_mode=DR)
            nc.tensor.matmul(g[:, P:2 * P], lh, lh,
                             start=False, stop=False, perf_mode=DR)
            nc.tensor.matmul(g[:, 2 * P:3 * P], le, le,
                             start=False, stop=last, perf_mode=DR)

        nc.vector.tensor_copy(os_[:], g[:])
        nc.sync.dma_start(out_d.ap(), os_[:])

    nc.compile()
    _MODULE_CACHE["nc"] = nc
    return nc


# ---------------------------------------------------------------- host side
def _shard_core(y_hat, y, core):
    """Per-core inputs -> X [128, 64, 2, 128] fp8 in (bc, blk, tensor, t)."""
    outs = []
    for arr in (y, y_hat):                              # tensor 0 = y, 1 = y_hat
        s = arr[core * BPC:(core + 1) * BPC]            # [16, T, 8]
        x = s.transpose(0, 2, 1).reshape(BC, NBLK, P)   # [bc, blk, t]
        outs.append(x.astype(FP8))
    return np.ascontiguousarray(np.stack(outs, axis=2))  # [bc, blk, 2, t]


def _emulate_core(x_core):
    """Numpy emulation of the device Grams for one core (validation aid)."""
    xe = x_core[:, :, 0, :].astype(np.float32)
    xh = x_core[:, :, 1, :].astype(np.float32)
    ghe = np.einsum("sju,sjv->uv", xh, xe, optimize=True)
    ghh = np.einsum("sju,sjv->uv", xh, xh, optimize=True)
    gee = np.einsum("sju,sjv->uv", xe, xe, optimize=True)
    return np.concatenate([ghe, ghh, gee], axis=1)       # [128, 384] f32


def _host_reduce(ghe, ghh, gee, y_hat, y):
    """Assemble the final scalar loss (f64) from summed Grams + raw inputs."""
    xh = y_hat.astype(np.float64)
    xe = y.astype(np.float64)

    rng = range(-S_BAND, S_BAND + 1)
    rt_he = {s: np.diagonal(ghe, offset=-s).sum() for s in rng}
    rt_hh = {s: np.diagonal(ghh, offset=-s).sum() for s in rng}
    rt_ee = {s: np.diagonal(gee, offset=-s).sum() for s in rng}

    def kker(m):
        return ALPHA * ALPHA * QD ** abs(m) / (1.0 - QD * QD)

    corr_inf = {d: sum(kker(s - d) * rt_he[s] for s in rng)
                for d in range(-KSH, KSH + 1)}
    shh_inf = sum(kker(s) * rt_hh[s] for s in rng)
    see_inf = sum(kker(s) * rt_ee[s] for s in rng)

    # --- exact head scans (stationary EMA, e_{-1} = 0) over [0, H_HEAD+8)
    def head_scan(x):
        e = np.zeros((B, C))
        out = []
        for t in range(H_HEAD + 8):
            e = ALPHA * x[:, t, :] + QD * e
            out.append(e.copy())
        return np.stack(out, axis=1)                     # [B, H+8, C]

    es_h_head = head_scan(xh)
    es_e_head = head_scan(xe)

    # --- tail scans: stationary EMA values for t in [T-16, T+EXT)
    def tail_scan(x):
        t0 = T - (TAIL_WARM + 16)
        e = np.zeros((B, C))
        keep = []
        for t in range(t0, T):
            e = ALPHA * x[:, t, :] + QD * e
            if t >= T - 16:
                keep.append(e.copy())
        arr = np.stack(keep, axis=1)                     # [B, 16, C]
        ext = arr[:, -1:, :] * (QD ** np.arange(1, EXT + 1))[None, :, None]
        return np.concatenate([arr, ext], axis=1)        # t = T-16 .. T+EXT-1

    es_h_tail = tail_scan(xh)
    es_e_tail = tail_scan(xe)

    def tail_at(arr, t):                                 # t >= T-16
        return arr[:, t - (T - 16), :]

    xh0 = xh[:, 0, :]
    xe0 = xe[:, 0, :]
    qpow = QD ** (np.arange(H_HEAD + 8) + 1.0)
    phi_h = qpow[None, :, None] * xh0[:, None, :]        # EMA-init correction
    phi_e = qpow[None, :, None] * xe0[:, None, :]
    etrue_h_head = es_h_head + phi_h
    etrue_e_head = es_e_head + phi_e

    geo = QD * QD / (1.0 - QD * QD)
    nh = (shh_inf - (tail_at(es_h_tail, T - 1) ** 2).sum() * geo
          + (2.0 * phi_h[:, :H_HEAD, :] * es_h_head[:, :H_HEAD, :]).sum()
          + (xh0 ** 2).sum() * geo)
    ne = (see_inf - (tail_at(es_e_tail, T - 1) ** 2).sum() * geo
          + (2.0 * phi_e[:, :H_HEAD, :] * es_e_head[:, :H_HEAD, :]).sum()
          + (xe0 ** 2).sum() * geo)

    corr_true = {}
    for d in range(-KSH, KSH + 1):
        # remove t >= Tlim terms of sum_t estat_h[t+d] estat_e[t]
        tlim = T - d if d >= 0 else T
        ts = np.arange(tlim, T + EXT - 16 - abs(d))
        rem = 0.0
        if len(ts):
            eh = np.stack([tail_at(es_h_tail, t + d) for t in ts], axis=1)
            ee = np.stack([tail_at(es_e_tail, t) for t in ts], axis=1)
            rem = (eh * ee).sum()
        # EMA-init (phi) cross terms over the true t range, truncated at H
        tcr = np.arange(max(0, -d), H_HEAD)
        ph = phi_h[:, tcr + d, :]
        pe = phi_e[:, tcr, :]
        esh = es_h_head[:, tcr + d, :]
        ese = es_e_head[:, tcr, :]
        corr_true[d] = corr_inf[d] - rem + (ph * ese + esh * pe + ph * pe).sum()

    head_eh = (etrue_h_head[:, :KSH, :] ** 2).sum(axis=(0, 2))
    head_ee = (etrue_e_head[:, :KSH, :] ** 2).sum(axis=(0, 2))
    tail_eh = np.array([(tail_at(es_h_tail, T - KSH + k) ** 2).sum()
                        for k in range(KSH)])
    tail_ee = np.array([(tail_at(es_e_tail, T - KSH + k) ** 2).sum()
                        for k in range(KSH)])

    errs = []
    for d in range(-KSH, KSH + 1):
        nd = B * C * (T - abs(d))
        if d >= 0:
            cut = head_eh[:d].sum() + tail_ee[KSH - d:].sum() if d > 0 else 0.0
        else:
            s = -d
            cut = head_ee[:s].sum() + tail_eh[KSH - s:].sum()
        errs.append(((nh + ne - cut) - 2.0 * corr_true[d]) / nd)

    db = (rt_hh[0] + rt_ee[0] - 2.0 * rt_he[0]) / (B * T * C)
    return db + LAM * min(errs)


def _run_device(y_hat, y, trace=False):
    from concourse.bass_utils import run_bass_kernel_spmd

    nc = _build_module()
    in_maps = [{"x": _shard_core(y_hat, y, core)} for core in range(NCORES)]
    return run_bass_kernel_spmd(
        nc, in_maps, core_ids=list(range(NCORES)), trace=trace,
    )


def _reduce_results(results, y_hat, y):
    ghe = np.zeros((P, P), np.float64)
    ghh = np.zeros((P, P), np.float64)
    gee = np.zeros((P, P), np.float64)
    for r in results:
        out = r["out"].astype(np.float64)
        ghe += out[:, 0:P]
        ghh += out[:, P:2 * P]
        gee += out[:, 2 * P:3 * P]
    return np.float32(_host_reduce(ghe, ghh, gee, y_hat, y))


def kernel(y_hat, y):
    res = _run_device(y_hat, y, trace=False)
    return _reduce_results(res.results, y_hat, y)


# revision 9
# speedup vs baseline: 2.6514x; 1.0177x over previous
"""Trainium2 Bass kernel for nn_DBLossWithShift.

Computes: mean((y_hat-y)^2) + 0.1 * min_{|d|<=5} mean((EMA(y_hat)[t+d]-EMA(y)[t])^2)
for y_hat, y of shape [128, 8192, 8] f32, EMA along t with alpha=0.2.

Key identity: the EMA is a linear filter, so every term of the loss is a
quadratic form in the raw inputs.  With the stationary kernel a[i] = a*q^i,

    sum_t E1[t+d]*E2[t]  =  sum_s K(s-d) * R_12(s),    K(m) = a^2 q^|m| / (1-q^2)

where R_12(s) = sum_t x1[t+s]*x2[t] is the raw lag-s cross-correlation.  The
device therefore only computes three block-diagonal Gram matrices of the raw
(fp8-quantized) inputs per core:

    G_12[u, v] = sum_{j<64} sum_{bc} x1[bc, 128j+u] * x2[bc, 128j+v]

for (x_hat,x), (x_hat,x_hat), (x,x).  Host sums the G's over cores, takes
diagonal sums R~(s), applies the K smoothing, and adds exact f64 head/tail
corrections (EMA init e_0 = x_0, range cuts, beyond-T decay) computed from
short scans of the raw inputs.  db_loss comes exactly from the s=0 diagonals.

Approximations (validated ~7.6e-4 total rel err vs reference, gate is 2e-2):
  - fp8_e4m3 input quantization (dominant: ~7.6e-4 on db)
  - cross-block lag pairs dropped from R~ (~1e-6)
  - K truncated at |m| <= S_BAND=96 (~1e-9)

Device schedule (per core): 9 input DMA chunks (fp8, 2 MiB total) overlapped
with 96 DoubleRow fp8 matmuls (K=256 via block pairs) accumulating the three
Grams in one PSUM bank; one DVE evacuation to bf16; one output DMA.
Data-parallel over batch: 16 batch elements per core, (b,c) -> 128 partitions.
"""

import sys

import numpy as np

for _p in ("/opt/trn_rl_repo",):
    if _p not in sys.path:
        sys.path.insert(0, _p)

import ml_dtypes

# ---------------------------------------------------------------- constants
B, T, C = 128, 8192, 8
NCORES = 8
BPC = B // NCORES          # 16 batch elements per core
BC = BPC * C               # 128 partitions (b*8 + c)
P = 128                    # t-block size
NBLK = T // P              # 64 blocks
ALPHA, QD = 0.2, 0.8
KSH = 5                    # max |shift|
LAM = 0.1

S_BAND = 96                # diagonal band of G used by the host reduction
H_HEAD = 192               # head-scan length for exact EMA-init corrections
EXT = 224                  # tail extension (decay) length
TAIL_WARM = 768            # tail-scan warmup steps

# input DMA chunk sizes in blocks (sum = 64, all even)
CHUNKS = (8, 8, 8, 8, 8, 8, 8, 6, 2)

FP8 = ml_dtypes.float8_e4m3    # TRN float8e4 (max +-240; N(0,1) data is safe)


# ---------------------------------------------------------------- device IR
_MODULE_CACHE = {}


def _build_module():
    if "nc" in _MODULE_CACHE:
        return _MODULE_CACHE["nc"]
    from contextlib import ExitStack

    import concourse.tile as tile
    from concourse import bacc, mybir

    f8 = mybir.dt.float8e4
    f32 = mybir.dt.float32
    bf16 = mybir.dt.bfloat16
    DR = mybir.MatmulPerfMode.DoubleRow

    nc = bacc.Bacc("TRN2", target_bir_lowering=False, debug=False)
    # X layout: [bc, blk, tensor, t] with tensor 0 = y (xe), 1 = y_hat (xh)
    x_d = nc.dram_tensor("x", [BC, NBLK, 2, P], f8, kind="ExternalInput")
    # out: [he | ss] Grams (ss = hh + ee, summed on device), bf16
    out_d = nc.dram_tensor("out", [P, 2 * P], bf16, kind="ExternalOutput")

    with tile.TileContext(nc) as tc, ExitStack() as ctx:
        xp = ctx.enter_context(tc.tile_pool(name="xin", bufs=1))
        op = ctx.enter_context(tc.tile_pool(name="outs", bufs=1))
        pp = ctx.enter_context(tc.tile_pool(name="pacc", bufs=1, space="PSUM"))

        xt = xp.tile([BC, NBLK, 2, P], f8, tag="xt")
        c0 = 0
        for i, cb in enumerate(CHUNKS):
            c1 = c0 + cb
            eng = nc.sync if i % 2 == 0 else nc.scalar
            eng.dma_start(xt[:, c0:c1], x_d.ap()[:, c0:c1])
            c0 = c1
        assert c0 == NBLK

        g = pp.tile([P, 2 * P], f32, tag="g")
        os_ = op.tile([P, 2 * P], bf16, tag="os")

        # One accumulation group for the whole PSUM bank: start only on the
        # very first matmul (marks the 2 KiB zero region pending-zero once),
        # stop only on the last.  Each matmul's 128-col slice is uniformly
        # virgin (pair 0: overwrite via has_written) or written (accumulate).
        # The he Gram pairs two consecutive t-blocks along DoubleRow's k-tile
        # dim; the ss Gram instead pairs the two tensors of one block, which
        # accumulates xh'xh + xe'xe in a single matmul.
        npair = NBLK // 2
        for m in range(npair):
            j0 = 2 * m
            lh = xt[:, j0:j0 + 2, 1, :]
            le = xt[:, j0:j0 + 2, 0, :]
            first = m == 0
            last = m == npair - 1
            nc.tensor.matmul(g[:, 0:P], lh, le,
                             start=first, stop=False, perf_mode=DR)
            nc.tensor.matmul(g[:, P:2 * P], xt[:, j0, :, :], xt[:, j0, :, :],
                             start=False, stop=False, perf_mode=DR)
            nc.tensor.matmul(g[:, P:2 * P], xt[:, j0 + 1, :, :], xt[:, j0 + 1, :, :],
                             start=False, stop=last, perf_mode=DR)

        nc.vector.tensor_copy(os_[:], g[:])
        nc.sync.dma_start(out_d.ap(), os_[:])

    nc.compile()
    _MODULE_CACHE["nc"] = nc
    return nc


# ---------------------------------------------------------------- host side
def _shard_core(y_hat, y, core):
    """Per-core inputs -> X [128, 64, 2, 128] fp8 in (bc, blk, tensor, t)."""
    outs = []
    for arr in (y, y_hat):                              # tensor 0 = y, 1 = y_hat
        s = arr[core * BPC:(core + 1) * BPC]            # [16, T, 8]
        x = s.transpose(0, 2, 1).reshape(BC, NBLK, P)   # [bc, blk, t]
        outs.append(x.astype(FP8))
    return np.ascontiguousarray(np.stack(outs, axis=2))  # [bc, blk, 2, t]


def _emulate_core(x_core):
    """Numpy emulation of the device Grams for one core (validation aid)."""
    xe = x_core[:, :, 0, :].astype(np.float32)
    xh = x_core[:, :, 1, :].astype(np.float32)
    ghe = np.einsum("sju,sjv->uv", xh, xe, optimize=True)
    gss = (np.einsum("sju,sjv->uv", xh, xh, optimize=True)
           + np.einsum("sju,sjv->uv", xe, xe, optimize=True))
    return np.concatenate([ghe, gss], axis=1)            # [128, 256] f32


def _host_reduce(ghe, gss, y_hat, y):
    """Assemble the final scalar loss (f64) from summed Grams + raw inputs."""
    xh = y_hat.astype(np.float64)
    xe = y.astype(np.float64)

    rng = range(-S_BAND, S_BAND + 1)
    rt_he = {s: np.diagonal(ghe, offset=-s).sum() for s in rng}
    rt_ss = {s: np.diagonal(gss, offset=-s).sum() for s in rng}

    def kker(m):
        return ALPHA * ALPHA * QD ** abs(m) / (1.0 - QD * QD)

    corr_inf = {d: sum(kker(s - d) * rt_he[s] for s in rng)
                for d in range(-KSH, KSH + 1)}
    sss_inf = sum(kker(s) * rt_ss[s] for s in rng)

    # --- exact head scans (stationary EMA, e_{-1} = 0) over [0, H_HEAD+8)
    def head_scan(x):
        e = np.zeros((B, C))
        out = []
        for t in range(H_HEAD + 8):
            e = ALPHA * x[:, t, :] + QD * e
            out.append(e.copy())
        return np.stack(out, axis=1)                     # [B, H+8, C]

    es_h_head = head_scan(xh)
    es_e_head = head_scan(xe)

    # --- tail scans: stationary EMA values for t in [T-16, T+EXT)
    def tail_scan(x):
        t0 = T - (TAIL_WARM + 16)
        e = np.zeros((B, C))
        keep = []
        for t in range(t0, T):
            e = ALPHA * x[:, t, :] + QD * e
            if t >= T - 16:
                keep.append(e.copy())
        arr = np.stack(keep, axis=1)                     # [B, 16, C]
        ext = arr[:, -1:, :] * (QD ** np.arange(1, EXT + 1))[None, :, None]
        return np.concatenate([arr, ext], axis=1)        # t = T-16 .. T+EXT-1

    es_h_tail = tail_scan(xh)
    es_e_tail = tail_scan(xe)

    def tail_at(arr, t):                                 # t >= T-16
        return arr[:, t - (T - 16), :]

    xh0 = xh[:, 0, :]
    xe0 = xe[:, 0, :]
    qpow = QD ** (np.arange(H_HEAD + 8) + 1.0)
    phi_h = qpow[None, :, None] * xh0[:, None, :]        # EMA-init correction
    phi_e = qpow[None, :, None] * xe0[:, None, :]
    etrue_h_head = es_h_head + phi_h
    etrue_e_head = es_e_head + phi_e

    geo = QD * QD / (1.0 - QD * QD)
    # nsum = NH + NE (the host reduction only ever needs their sum)
    nsum = (sss_inf
            - ((tail_at(es_h_tail, T - 1) ** 2).sum()
               + (tail_at(es_e_tail, T - 1) ** 2).sum()) * geo
            + (2.0 * phi_h[:, :H_HEAD, :] * es_h_head[:, :H_HEAD, :]).sum()
            + (2.0 * phi_e[:, :H_HEAD, :] * es_e_head[:, :H_HEAD, :]).sum()
            + ((xh0 ** 2).sum() + (xe0 ** 2).sum()) * geo)

    corr_true = {}
    for d in range(-KSH, KSH + 1):
        # remove t >= Tlim terms of sum_t estat_h[t+d] estat_e[t]
        tlim = T - d if d >= 0 else T
        ts = np.arange(tlim, T + EXT - 16 - abs(d))
        rem = 0.0
        if len(ts):
            eh = np.stack([tail_at(es_h_tail, t + d) for t in ts], axis=1)
            ee = np.stack([tail_at(es_e_tail, t) for t in ts], axis=1)
            rem = (eh * ee).sum()
        # EMA-init (phi) cross terms over the true t range, truncated at H
        tcr = np.arange(max(0, -d), H_HEAD)
        ph = phi_h[:, tcr + d, :]
        pe = phi_e[:, tcr, :]
        esh = es_h_head[:, tcr + d, :]
        ese = es_e_head[:, tcr, :]
        corr_true[d] = corr_inf[d] - rem + (ph * ese + esh * pe + ph * pe).sum()

    head_eh = (etrue_h_head[:, :KSH, :] ** 2).sum(axis=(0, 2))
    head_ee = (etrue_e_head[:, :KSH, :] ** 2).sum(axis=(0, 2))
    tail_eh = np.array([(tail_at(es_h_tail, T - KSH + k) ** 2).sum()
                        for k in range(KSH)])
    tail_ee = np.array([(tail_at(es_e_tail, T - KSH + k) ** 2).sum()
                        for k in range(KSH)])

    errs = []
    for d in range(-KSH, KSH + 1):
        nd = B * C * (T - abs(d))
        if d >= 0:
            cut = head_eh[:d].sum() + tail_ee[KSH - d:].sum() if d > 0 else 0.0
        else:
            s = -d
            cut = head_ee[:s].sum() + tail_eh[KSH - s:].sum()
        errs.append(((nsum - cut) - 2.0 * corr_true[d]) / nd)

    db = (rt_ss[0] - 2.0 * rt_he[0]) / (B * T * C)
    return db + LAM * min(errs)


def _run_device(y_hat, y, trace=False):
    from concourse.bass_utils import run_bass_kernel_spmd

    nc = _build_module()
    in_maps = [{"x": _shard_core(y_hat, y, core)} for core in range(NCORES)]
    return run_bass_kernel_spmd(
        nc, in_maps, core_ids=list(range(NCORES)), trace=trace,
    )


def _reduce_results(results, y_hat, y):
    ghe = np.zeros((P, P), np.float64)
    gss = np.zeros((P, P), np.float64)
    for r in results:
        out = r["out"].astype(np.float64)
        ghe += out[:, 0:P]
        gss += out[:, P:2 * P]
    return np.float32(_host_reduce(ghe, gss, y_hat, y))


def kernel(y_hat, y):
    res = _run_device(y_hat, y, trace=False)
    return _reduce_results(res.results, y_hat, y)


# revision 10
# speedup vs baseline: 2.6723x; 1.0079x over previous
"""Trainium2 Bass kernel for nn_DBLossWithShift.

Computes: mean((y_hat-y)^2) + 0.1 * min_{|d|<=5} mean((EMA(y_hat)[t+d]-EMA(y)[t])^2)
for y_hat, y of shape [128, 8192, 8] f32, EMA along t with alpha=0.2.

Key identity: the EMA is a linear filter, so every term of the loss is a
quadratic form in the raw inputs.  With the stationary kernel a[i] = a*q^i,

    sum_t E1[t+d]*E2[t]  =  sum_s K(s-d) * R_12(s),    K(m) = a^2 q^|m| / (1-q^2)

where R_12(s) = sum_t x1[t+s]*x2[t] is the raw lag-s cross-correlation.  The
device therefore only computes three block-diagonal Gram matrices of the raw
(fp8-quantized) inputs per core:

    G_12[u, v] = sum_{j<64} sum_{bc} x1[bc, 128j+u] * x2[bc, 128j+v]

for (x_hat,x), (x_hat,x_hat), (x,x).  Host sums the G's over cores, takes
diagonal sums R~(s), applies the K smoothing, and adds exact f64 head/tail
corrections (EMA init e_0 = x_0, range cuts, beyond-T decay) computed from
short scans of the raw inputs.  db_loss comes exactly from the s=0 diagonals.

Approximations (validated ~7.6e-4 total rel err vs reference, gate is 2e-2):
  - fp8_e4m3 input quantization (dominant: ~7.6e-4 on db)
  - cross-block lag pairs dropped from R~ (~1e-6)
  - K truncated at |m| <= S_BAND=96 (~1e-9)

Device schedule (per core): 9 input DMA chunks (fp8, 2 MiB total) overlapped
with 96 DoubleRow fp8 matmuls (K=256 via block pairs) accumulating the three
Grams in one PSUM bank; one DVE evacuation to bf16; one output DMA.
Data-parallel over batch: 16 batch elements per core, (b,c) -> 128 partitions.
"""

import sys

import numpy as np

for _p in ("/opt/trn_rl_repo",):
    if _p not in sys.path:
        sys.path.insert(0, _p)

import ml_dtypes

# ---------------------------------------------------------------- constants
B, T, C = 128, 8192, 8
NCORES = 8
BPC = B // NCORES          # 16 batch elements per core
BC = BPC * C               # 128 partitions (b*8 + c)
P = 128                    # t-block size
NBLK = T // P              # 64 blocks
ALPHA, QD = 0.2, 0.8
KSH = 5                    # max |shift|
LAM = 0.1

S_BAND = 96                # diagonal band of G used by the host reduction
H_HEAD = 192               # head-scan length for exact EMA-init corrections
EXT = 224                  # tail extension (decay) length
TAIL_WARM = 768            # tail-scan warmup steps

# input DMA chunk sizes in blocks (sum = 64, all even)
CHUNKS = (10, 8, 8, 8, 8, 8, 8, 4, 2)

FP8 = ml_dtypes.float8_e4m3    # TRN float8e4 (max +-240; N(0,1) data is safe)


# ---------------------------------------------------------------- device IR
_MODULE_CACHE = {}


def _build_module():
    if "nc" in _MODULE_CACHE:
        return _MODULE_CACHE["nc"]
    from contextlib import ExitStack

    import concourse.tile as tile
    from concourse import bacc, mybir

    f8 = mybir.dt.float8e4
    f32 = mybir.dt.float32
    bf16 = mybir.dt.bfloat16
    DR = mybir.MatmulPerfMode.DoubleRow

    nc = bacc.Bacc("TRN2", target_bir_lowering=False, debug=False)
    # X layout: [bc, blk, tensor, t] with tensor 0 = y (xe), 1 = y_hat (xh)
    x_d = nc.dram_tensor("x", [BC, NBLK, 2, P], f8, kind="ExternalInput")
    # out: [he | ss] Grams (ss = hh + ee, summed on device), bf16
    out_d = nc.dram_tensor("out", [P, 2 * P], bf16, kind="ExternalOutput")

    with tile.TileContext(nc) as tc, ExitStack() as ctx:
        xp = ctx.enter_context(tc.tile_pool(name="xin", bufs=1))
        op = ctx.enter_context(tc.tile_pool(name="outs", bufs=1))
        pp = ctx.enter_context(tc.tile_pool(name="pacc", bufs=1, space="PSUM"))

        xt = xp.tile([BC, NBLK, 2, P], f8, tag="xt")
        c0 = 0
        for i, cb in enumerate(CHUNKS):
            c1 = c0 + cb
            eng = nc.sync if i % 2 == 0 else nc.scalar
            eng.dma_start(xt[:, c0:c1], x_d.ap()[:, c0:c1])
            c0 = c1
        assert c0 == NBLK

        g = pp.tile([P, 2 * P], f32, tag="g")
        os_ = op.tile([P, 2 * P], bf16, tag="os")

        # One accumulation group for the whole PSUM bank: start only on the
        # very first matmul (marks the 2 KiB zero region pending-zero once),
        # stop only on the last.  Each matmul's 128-col slice is uniformly
        # virgin (pair 0: overwrite via has_written) or written (accumulate).
        # The he Gram pairs two consecutive t-blocks along DoubleRow's k-tile
        # dim; the ss Gram instead pairs the two tensors of one block, which
        # accumulates xh'xh + xe'xe in a single matmul.
        npair = NBLK // 2
        for m in range(npair):
            j0 = 2 * m
            lh = xt[:, j0:j0 + 2, 1, :]
            le = xt[:, j0:j0 + 2, 0, :]
            first = m == 0
            last = m == npair - 1
            nc.tensor.matmul(g[:, 0:P], lh, le,
                             start=first, stop=False, perf_mode=DR)
            nc.tensor.matmul(g[:, P:2 * P], xt[:, j0, :, :], xt[:, j0, :, :],
                             start=False, stop=False, perf_mode=DR)
            nc.tensor.matmul(g[:, P:2 * P], xt[:, j0 + 1, :, :], xt[:, j0 + 1, :, :],
                             start=False, stop=last, perf_mode=DR)

        nc.vector.tensor_copy(os_[:], g[:])
        nc.sync.dma_start(out_d.ap(), os_[:])

    nc.compile()
    _MODULE_CACHE["nc"] = nc
    return nc


# ---------------------------------------------------------------- host side
def _shard_core(y_hat, y, core):
    """Per-core inputs -> X [128, 64, 2, 128] fp8 in (bc, blk, tensor, t)."""
    outs = []
    for arr in (y, y_hat):                              # tensor 0 = y, 1 = y_hat
        s = arr[core * BPC:(core + 1) * BPC]            # [16, T, 8]
        x = s.transpose(0, 2, 1).reshape(BC, NBLK, P)   # [bc, blk, t]
        outs.append(x.astype(FP8))
    return np.ascontiguousarray(np.stack(outs, axis=2))  # [bc, blk, 2, t]


def _emulate_core(x_core):
    """Numpy emulation of the device Grams for one core (validation aid)."""
    xe = x_core[:, :, 0, :].astype(np.float32)
    xh = x_core[:, :, 1, :].astype(np.float32)
    ghe = np.einsum("sju,sjv->uv", xh, xe, optimize=True)
    gss = (np.einsum("sju,sjv->uv", xh, xh, optimize=True)
           + np.einsum("sju,sjv->uv", xe, xe, optimize=True))
    return np.concatenate([ghe, gss], axis=1)            # [128, 256] f32


def _host_reduce(ghe, gss, y_hat, y):
    """Assemble the final scalar loss (f64) from summed Grams + raw inputs."""
    xh = y_hat.astype(np.float64)
    xe = y.astype(np.float64)

    rng = range(-S_BAND, S_BAND + 1)
    rt_he = {s: np.diagonal(ghe, offset=-s).sum() for s in rng}
    rt_ss = {s: np.diagonal(gss, offset=-s).sum() for s in rng}

    def kker(m):
        return ALPHA * ALPHA * QD ** abs(m) / (1.0 - QD * QD)

    corr_inf = {d: sum(kker(s - d) * rt_he[s] for s in rng)
                for d in range(-KSH, KSH + 1)}
    sss_inf = sum(kker(s) * rt_ss[s] for s in rng)

    # --- exact head scans (stationary EMA, e_{-1} = 0) over [0, H_HEAD+8)
    def head_scan(x):
        e = np.zeros((B, C))
        out = []
        for t in range(H_HEAD + 8):
            e = ALPHA * x[:, t, :] + QD * e
            out.append(e.copy())
        return np.stack(out, axis=1)                     # [B, H+8, C]

    es_h_head = head_scan(xh)
    es_e_head = head_scan(xe)

    # --- tail scans: stationary EMA values for t in [T-16, T+EXT)
    def tail_scan(x):
        t0 = T - (TAIL_WARM + 16)
        e = np.zeros((B, C))
        keep = []
        for t in range(t0, T):
            e = ALPHA * x[:, t, :] + QD * e
            if t >= T - 16:
                keep.append(e.copy())
        arr = np.stack(keep, axis=1)                     # [B, 16, C]
        ext = arr[:, -1:, :] * (QD ** np.arange(1, EXT + 1))[None, :, None]
        return np.concatenate([arr, ext], axis=1)        # t = T-16 .. T+EXT-1

    es_h_tail = tail_scan(xh)
    es_e_tail = tail_scan(xe)

    def tail_at(arr, t):                                 # t >= T-16
        return arr[:, t - (T - 16), :]

    xh0 = xh[:, 0, :]
    xe0 = xe[:, 0, :]
    qpow = QD ** (np.arange(H_HEAD + 8) + 1.0)
    phi_h = qpow[None, :, None] * xh0[:, None, :]        # EMA-init correction
    phi_e = qpow[None, :, None] * xe0[:, None, :]
    etrue_h_head = es_h_head + phi_h
    etrue_e_head = es_e_head + phi_e

    geo = QD * QD / (1.0 - QD * QD)
    # nsum = NH + NE (the host reduction only ever needs their sum)
    nsum = (sss_inf
            - ((tail_at(es_h_tail, T - 1) ** 2).sum()
               + (tail_at(es_e_tail, T - 1) ** 2).sum()) * geo
            + (2.0 * phi_h[:, :H_HEAD, :] * es_h_head[:, :H_HEAD, :]).sum()
            + (2.0 * phi_e[:, :H_HEAD, :] * es_e_head[:, :H_HEAD, :]).sum()
            + ((xh0 ** 2).sum() + (xe0 ** 2).sum()) * geo)

    corr_true = {}
    for d in range(-KSH, KSH + 1):
        # remove t >= Tlim terms of sum_t estat_h[t+d] estat_e[t]
        tlim = T - d if d >= 0 else T
        ts = np.arange(tlim, T + EXT - 16 - abs(d))
        rem = 0.0
        if len(ts):
            eh = np.stack([tail_at(es_h_tail, t + d) for t in ts], axis=1)
            ee = np.stack([tail_at(es_e_tail, t) for t in ts], axis=1)
            rem = (eh * ee).sum()
        # EMA-init (phi) cross terms over the true t range, truncated at H
        tcr = np.arange(max(0, -d), H_HEAD)
        ph = phi_h[:, tcr + d, :]
        pe = phi_e[:, tcr, :]
        esh = es_h_head[:, tcr + d, :]
        ese = es_e_head[:, tcr, :]
        corr_true[d] = corr_inf[d] - rem + (ph * ese + esh * pe + ph * pe).sum()

    head_eh = (etrue_h_head[:, :KSH, :] ** 2).sum(axis=(0, 2))
    head_ee = (etrue_e_head[:, :KSH, :] ** 2).sum(axis=(0, 2))
    tail_eh = np.array([(tail_at(es_h_tail, T - KSH + k) ** 2).sum()
                        for k in range(KSH)])
    tail_ee = np.array([(tail_at(es_e_tail, T - KSH + k) ** 2).sum()
                        for k in range(KSH)])

    errs = []
    for d in range(-KSH, KSH + 1):
        nd = B * C * (T - abs(d))
        if d >= 0:
            cut = head_eh[:d].sum() + tail_ee[KSH - d:].sum() if d > 0 else 0.0
        else:
            s = -d
            cut = head_ee[:s].sum() + tail_eh[KSH - s:].sum()
        errs.append(((nsum - cut) - 2.0 * corr_true[d]) / nd)

    db = (rt_ss[0] - 2.0 * rt_he[0]) / (B * T * C)
    return db + LAM * min(errs)


def _run_device(y_hat, y, trace=False):
    from concourse.bass_utils import run_bass_kernel_spmd

    nc = _build_module()
    in_maps = [{"x": _shard_core(y_hat, y, core)} for core in range(NCORES)]
    return run_bass_kernel_spmd(
        nc, in_maps, core_ids=list(range(NCORES)), trace=trace,
    )


def _reduce_results(results, y_hat, y):
    ghe = np.zeros((P, P), np.float64)
    gss = np.zeros((P, P), np.float64)
    for r in results:
        out = r["out"].astype(np.float64)
        ghe += out[:, 0:P]
        gss += out[:, P:2 * P]
    return np.float32(_host_reduce(ghe, gss, y_hat, y))


def kernel(y_hat, y):
    res = _run_device(y_hat, y, trace=False)
    return _reduce_results(res.results, y_hat, y)
